# revision 1
# baseline (speedup 1.0000x reference)
"""Trainium2 Bass kernel for nn_Network_85220741087986.

3-layer MLP: per layer  X[N,1024] @ W[1024,2048]  ->  per-group bilinear
interpolation activation on a 3x3 grid (ARITY=2; output group g consumes
the feature pair (2g, 2g+1)) -> X[N,1024].

Reformulation (validated exact vs the reference on host): the multilinear
interpolation over the per-axis grid {-1,0,1} equals
    result = sum_{r,c} K[r,c] * f0_r * f1_c,   f = (1, u, v'),
    u = clip(x,0,1), v' = clip(x,-1,0),
with K a per-group 3x3 constant matrix precomputed on host from the
activation table (K = B M B^T, sign-folded for v').  K00 (the constant
term) of layer i is folded into layer i+1's pre-activation bias host-side;
the last layer's K00 + out_bias rides the row0 affine.

On-chip layout: features on partitions, samples on the free dim.  Host
pre-transposes X and reorders W columns (f in [0,1024) = a=0 features,
f in [1024,2048) = a=1), so no transposes are needed on-chip and each
activation output tile [128 groups, S samples] is directly the next
layer's matmul rhs.

Per group-tile block (128 groups x 2048 samples):
  PE : 2 f-tiles x 4 psum-chunks x 8 k-tiles fp16 matmuls -> PSUM fp32
  ACT: 8 psum->SBUF fp16 copies with per-feature bias (Identity+bias AP),
       3 row affines  a_r = u1*K_r1 + K_r0
  DVE: 4 clips (tensor_scalar max/min), 3 row affines b_r = v1*K_r2',
       7 tensor_tensor (row adds, u0/v0 products, final sums)

Sharding: pure data parallel over 8 cores (2048 samples each), weights
and tables replicated.  fp16 on-chip tensors, fp32 PSUM/constants/output.
Host-validated end-to-end error vs fp32 reference: ~2.3e-4 (norm rel).
"""

import os
import sys

import numpy as np

for _p in ("/opt/trn_rl_repo", "/root/.axon_site/_ro/trn_rl_repo"):
    if os.path.isdir(_p) and _p not in sys.path:
        sys.path.append(_p)

import concourse.bacc as bacc
import concourse.bass as bass
import concourse.mybir as mybir
import concourse.tile as tile
from concourse.alu_op_type import AluOpType
from concourse.bass_utils import run_bass_kernel_spmd


def _ensure_axon_hooks():
    """This image's antenv lacks axon_hooks; provide it (and register the
    NTFF profile hook) so trace=True doesn't crash run_bass_kernel_spmd."""
    import types

    try:
        import antenv.axon_hooks  # noqa: F401
        return
    except ImportError:
        pass
    mod = types.ModuleType("antenv.axon_hooks")
    _hook = [None]
    mod.get_axon_ntff_profile_hook = lambda: _hook[0]
    mod.set_axon_ntff_profile_hook = lambda h: _hook.__setitem__(0, h)
    sys.modules["antenv.axon_hooks"] = mod
    try:
        import antenv
        antenv.axon_hooks = mod
    except ImportError:
        pass
    try:
        from trn_agent_boot.trn_boot import _ntff_profile_via_ctypes
        so = "/opt/axon/libaxon_pjrt.so"
        if os.path.exists(so):
            _hook[0] = _ntff_profile_via_ctypes(so)
    except Exception:
        pass


_ensure_axon_hooks()

N_TOTAL = 16384
D_IN = 1024          # input features per layer
F_OUT = 2048         # matmul output features per layer (= 2 * groups)
N_LAYERS = 3
N_CORES = 8
NS = N_TOTAL // N_CORES   # samples per core
SC = NS                   # sample chunk = whole shard
KT = D_IN // 128          # 8 contraction tiles
FT = F_OUT // 128         # 16 matmul-output partition tiles
GT = D_IN // 128          # 8 group tiles
PS = 512                  # psum free dim (one fp32 bank)
F16 = mybir.dt.float16
F32 = mybir.dt.float32
AF = mybir.ActivationFunctionType

# consts tile column layout: kc0 kc1 kc2 (64 each) | b0 b1 b2 (16) | fb (8)
KC_OFF = [0, 64, 128]
B_OFF = [192, 208, 224]
FB_OFF = 240
C_COLS = 248

LAST_RESULTS = None  # BassKernelResults of the most recent run (for test.py)


def _prepare(inputs):
    """Host-side preprocessing -> per-core DRAM input arrays."""
    inp = {k: np.asarray(v) for k, v in inputs.items()}
    scale = float(np.abs(inp["scale"])) * 1.0  # SCALE_FACTOR = 1.0
    layer_scale = scale ** (1.0 / N_LAYERS)
    B = np.array([[0.0, 1, 0], [0, -1, 1], [1, -1, 0]])
    S = np.diag([1.0, 1.0, -1.0])
    host = {}
    consts = np.zeros((128, C_COLS), np.float32)
    prev_k00 = None
    for i in range(N_LAYERS):
        wpn = inp[f"w{i}"].astype(np.float64)
        raw_w = (wpn[:D_IN] - wpn[D_IN:]) * layer_scale       # [1024, 2048]
        # reorder columns: [:, :1024] = a=0 (x0 of group g), [:, 1024:] = a=1
        w_re = np.concatenate([raw_w[:, 0::2], raw_w[:, 1::2]], axis=1)
        bias = np.concatenate([inp[f"b{i}"][0::2], inp[f"b{i}"][1::2]]).astype(
            np.float64
        ) * 1.0  # BIAS_FACTOR = 1.0
        if prev_k00 is not None:
            bias = bias + prev_k00 @ w_re   # fold previous layer's K00
        a = inp[f"a{i}"].astype(np.float64)[:, :, 0]          # [1024, 9]
        m_ij = np.transpose(a.reshape(D_IN, 3, 3), (0, 2, 1))  # [g, i(ax0), j(ax1)]
        K = np.einsum("ri,gij,cj->grc", B, m_ij, B)           # [g, 3, 3]
        Kp = np.einsum("rs,gst,tc->grc", S, K, S)             # sign-fold for v'
        prev_k00 = K[:, 0, 0].copy()
        # per-group constants, order: K01 K02' K10 K11 K12' K20' K21' K22'
        kv = np.stack(
            [Kp[:, 0, 1], Kp[:, 0, 2], Kp[:, 1, 0], Kp[:, 1, 1],
             Kp[:, 1, 2], Kp[:, 2, 0], Kp[:, 2, 1], Kp[:, 2, 2]],
            axis=1,
        )  # [1024, 8]
        host[f"w{i}"] = np.ascontiguousarray(w_re.astype(np.float16))
        consts[:, KC_OFF[i]:KC_OFF[i] + 64] = (
            kv.reshape(GT, 128, 8).transpose(1, 0, 2).reshape(128, GT * 8)
        )
        consts[:, B_OFF[i]:B_OFF[i] + FT] = bias.reshape(FT, 128).T
    fb = prev_k00 + inp["out_bias"].astype(np.float64)
    consts[:, FB_OFF:FB_OFF + GT] = fb.reshape(GT, 128).T
    host["consts"] = np.ascontiguousarray(consts)
    x_t = np.ascontiguousarray(inp["X"].astype(np.float16).T)  # [1024, 16384]
    return host, x_t


def _build():
    nc = bacc.Bacc("TRN2", debug=False)
    x_d = nc.dram_tensor("xT", [D_IN, NS], F16, kind="ExternalInput")
    w_d = [nc.dram_tensor(f"w{i}", [D_IN, F_OUT], F16, kind="ExternalInput")
           for i in range(N_LAYERS)]
    c_d = nc.dram_tensor("consts", [128, C_COLS], F32, kind="ExternalInput")
    out_d = nc.dram_tensor("outT", [D_IN, NS], F16, kind="ExternalOutput")

    with tile.TileContext(nc) as tc:
        with (
            tc.tile_pool(name="w", bufs=1) as wpool,
            tc.tile_pool(name="c", bufs=1) as cpool,
            tc.tile_pool(name="x", bufs=1) as xpool,
            tc.tile_pool(name="pre", bufs=2) as prepool,
            tc.tile_pool(name="t", bufs=1) as tpool,
            tc.tile_pool(name="o", bufs=2) as opool,
            tc.tile_pool(name="ps", bufs=8, space="PSUM") as pspool,
        ):
            # constants: one DMA
            ct = cpool.tile([128, C_COLS], F32, tag="c")
            nc.sync.dma_start(ct[:], c_d[:])

            # warm-up activation: the first ACT instruction carries the fused
            # table load; keep it off the hot path and nearly dependency-free.
            warm = cpool.tile([128, 1], F32, tag="warm")
            nc.vector.memset(warm[:], 0.0)
            nc.scalar.activation(warm[:], warm[:], AF.Identity, bias=0.0,
                                 scale=1.0)

            # weights: one big tile per layer parity.
            # layout [128, KT*F_OUT]: k-tile kt at columns [kt*F_OUT, +F_OUT)
            def load_w(layer, split=False):
                t = wpool.tile([128, KT * F_OUT], F16, tag=f"w{layer % 2}")
                tv = t[:].rearrange("p (k f) -> p k f", k=KT)
                dv = w_d[layer][:].rearrange("(k p) f -> p k f", p=128)
                if split:
                    # one DMA per f-tile, issued in first-use order, so the
                    # first matmuls start after ~0.5MB instead of 4.2MB
                    for g in range(GT):
                        for half in range(2):
                            ft = g + GT * half
                            sl = slice(ft * 128, (ft + 1) * 128)
                            nc.sync.dma_start(tv[:, :, sl], dv[:, :, sl])
                else:
                    nc.sync.dma_start(tv, dv)
                return t

            w_cur = load_w(0, split=True)

            # input x tiles (one per k-tile) on the ACT HWDGE ring so they
            # stream concurrently with the weight DMAs on the sync ring;
            # first halves (the first psum chunks) first.
            x_tiles = {}
            xin0 = [xpool.tile([128, SC], F16, tag=f"xA{kt}",
                               name=f"xinA{kt}")
                    for kt in range(KT)]
            for h in range(2):
                for kt in range(KT):
                    nc.scalar.dma_start(
                        xin0[kt][:, h * (SC // 2):(h + 1) * (SC // 2)],
                        x_d[kt * 128:(kt + 1) * 128,
                            h * (SC // 2):(h + 1) * (SC // 2)],
                    )
            x_tiles[0] = xin0

            w_next = load_w(1)

            for layer in range(N_LAYERS):
                in_par = layer % 2
                out_par = (layer + 1) % 2
                last = layer == N_LAYERS - 1
                kco = KC_OFF[layer]
                xin = x_tiles[in_par]
                if not last:
                    xout = []
                for g in range(GT):
                    # --- matmuls for feature tiles g (x0) and g+8 (x1)
                    pre = []
                    for half in range(2):
                        ft = g + GT * half
                        pt = prepool.tile([128, SC], F16, tag=f"pre{half}")
                        for pc in range(SC // PS):
                            ps = pspool.tile([128, PS], F32, tag="ps")
                            for kt in range(KT):
                                nc.tensor.matmul(
                                    ps[:],
                                    w_cur[:, kt * F_OUT + ft * 128:
                                          kt * F_OUT + (ft + 1) * 128],
                                    xin[kt][:, pc * PS:(pc + 1) * PS],
                                    start=(kt == 0),
                                    stop=(kt == KT - 1),
                                )
                            # PSUM -> SBUF fp16 with per-feature bias
                            nc.scalar.activation(
                                pt[:, pc * PS:(pc + 1) * PS], ps[:],
                                AF.Identity,
                                bias=ct[:, B_OFF[layer] + ft:
                                        B_OFF[layer] + ft + 1],
                                scale=1.0,
                            )
                        pre.append(pt)
                    x0t, x1t = pre

                    # --- activation for group tile g
                    k = lambda j: ct[:, kco + g * 8 + j:kco + g * 8 + j + 1]
                    # clips (DVE, fp16 SBUF 4x)
                    v1 = tpool.tile([128, SC], F16, tag="v1")
                    nc.vector.tensor_scalar(
                        v1[:], x1t[:], 0.0, -1.0, AluOpType.min, AluOpType.max)
                    u1 = tpool.tile([128, SC], F16, tag="u1")
                    nc.vector.tensor_scalar(
                        u1[:], x1t[:], 0.0, 1.0, AluOpType.max, AluOpType.min)
                    u0 = tpool.tile([128, SC], F16, tag="u0")
                    nc.vector.tensor_scalar(
                        u0[:], x0t[:], 0.0, 1.0, AluOpType.max, AluOpType.min)
                    v0 = tpool.tile([128, SC], F16, tag="v0")
                    nc.vector.tensor_scalar(
                        v0[:], x0t[:], 0.0, -1.0, AluOpType.min, AluOpType.max)
                    # rows: a_r = u1*K_r1 + K_r0 (ACT), b_r = v1*K_r2 (DVE),
                    # row_r = a_r + b_r (DVE TT, in-place into b_r)
                    rows = []
                    for r, (kmul, kbias, kvmul) in enumerate(
                        [(0, None, 1), (3, 2, 4), (6, 5, 7)]
                    ):
                        nb = 2 if r == 0 else None
                        a_r = tpool.tile([128, SC], F16, tag=f"a{r}", bufs=nb)
                        if kbias is None:
                            bias_ap = (ct[:, FB_OFF + g:FB_OFF + g + 1]
                                       if last else 0.0)
                        else:
                            bias_ap = k(kbias)
                        nc.scalar.activation(
                            a_r[:], u1[:], AF.Identity, bias=bias_ap,
                            scale=k(kmul))
                        b_r = tpool.tile([128, SC], F16, tag=f"b{r}", bufs=nb)
                        nc.vector.tensor_scalar_mul(b_r[:], v1[:], k(kvmul))
                        nc.vector.tensor_tensor(
                            b_r[:], a_r[:], b_r[:], AluOpType.add)
                        rows.append(b_r)
                    row0, row1, row2 = rows
                    # products and final sum (DVE, in-place chains)
                    nc.vector.tensor_tensor(
                        row1[:], u0[:], row1[:], AluOpType.mult)
                    nc.vector.tensor_tensor(
                        row2[:], v0[:], row2[:], AluOpType.mult)
                    nc.vector.tensor_tensor(
                        row1[:], row1[:], row2[:], AluOpType.add)
                    if last:
                        # fp16 output (host upcasts): keeps the final TT in
                        # the DVE 2x mode and halves the store DMA
                        ot = opool.tile([128, SC], F16, tag="ostage")
                        nc.vector.tensor_tensor(
                            ot[:], row1[:], row0[:], AluOpType.add)
                        nc.sync.dma_start(
                            out_d[g * 128:(g + 1) * 128, :], ot[:])
                    else:
                        nt = xpool.tile(
                            [128, SC], F16,
                            tag=f"x{'A' if out_par == 0 else 'B'}{g}")
                        nc.vector.tensor_tensor(
                            nt[:], row1[:], row0[:], AluOpType.add)
                        xout.append(nt)
                if not last:
                    x_tiles[out_par] = xout
                if layer + 2 < N_LAYERS:
                    w_cur, w_next = w_next, load_w(layer + 2)
                else:
                    w_cur = w_next
    nc.compile()
    return nc


_NC_CACHE = None


def _get_nc():
    global _NC_CACHE
    if _NC_CACHE is None:
        _NC_CACHE = _build()
    return _NC_CACHE


def kernel(**inputs):
    global LAST_RESULTS
    host, x_t = _prepare(inputs)
    nc = _get_nc()
    in_maps = []
    for core in range(N_CORES):
        m = dict(host)
        m["xT"] = np.ascontiguousarray(x_t[:, core * NS:(core + 1) * NS])
        in_maps.append(m)
    want_trace = bool(os.environ.get("BASS_TRACE"))
    # First (untraced) run computes the result and initializes the PJRT
    # client; the profiler can only attach once that init has happened.
    os.environ["BASS_NEVER_TRACE"] = "1"
    try:
        res = run_bass_kernel_spmd(
            nc, in_maps, core_ids=list(range(N_CORES)), trace=False
        )
    finally:
        del os.environ["BASS_NEVER_TRACE"]
    LAST_RESULTS = res
    if want_trace:
        try:
            LAST_RESULTS = run_bass_kernel_spmd(
                nc, in_maps, core_ids=list(range(N_CORES)), trace=True
            )
        except Exception as e:  # profiling is best-effort
            print("trace run failed:", e)
    out_t = np.concatenate([r["outT"] for r in res.results], axis=1)  # [1024, N]
    return np.ascontiguousarray(out_t.T).astype(np.float32)

